# revision 1
# baseline (speedup 1.0000x reference)
"""MLA-style attention (nn_Attention_15496242004691) on 8 trn2 NeuronCores.

Strategy:
  Launch 1 (token-sharded, 512 tokens/core): cq = x@wq_a.T, ckv = x@wkv_a.T,
    RMSNorm of both (norm weights folded into the B projections on host),
    RoPE of k_pe (pair-swap folded into an extended wkv_a on host).
  Launch 2 (head-sharded, 2 heads/core): B projections (+q RoPE), causal
    attention with transposed scores (softmax sums via ones-matmul),
    output projection; host sums the 8 partial outputs.

All matmuls run as float32r (single-pass fp32, ~1.5e-4 rel err).
Activations are kept feature-on-partition so no on-chip transposes occur.
"""

import numpy as np

import concourse.bass as bass
import concourse.mybir as mybir
import concourse.tile as tile
from concourse import bacc
from concourse.bass_utils import run_bass_kernel_spmd

F32 = mybir.dt.float32
F32R = mybir.dt.float32r
AF = mybir.ActivationFunctionType
OP = mybir.AluOpType

B, S, DIM, H = 2, 2048, 2048, 16
NCORES = 8
HPC = H // NCORES  # heads per core = 2
RQ = RKV = 512
DN, DR, DV, DQK = 128, 64, 128, 192
EPS = 1e-6
SCALE = DQK ** -0.5
T = B * S          # 4096 tokens
TS = T // NCORES   # 512 tokens per core in launch 1

_CACHE = {}


# --------------------------------------------------------------------------
# Launch 1: A-projections + RMSNorm + k_pe RoPE (token-sharded)
# --------------------------------------------------------------------------
def build_k1():
    nc = bacc.Bacc("TRN2", target_bir_lowering=False)
    xt = nc.dram_tensor("xt", [DIM, TS], F32R, kind="ExternalInput")
    wqa = nc.dram_tensor("wqa", [128, 16, RQ], F32R, kind="ExternalInput")
    wkva = nc.dram_tensor("wkva", [128, 16, RKV + 2 * DR], F32R, kind="ExternalInput")
    cosk = nc.dram_tensor("cosk", [DR, TS], F32, kind="ExternalInput")
    sink = nc.dram_tensor("sink", [DR, TS], F32, kind="ExternalInput")
    onec = nc.dram_tensor("onec", [128, 1], F32R, kind="ExternalInput")
    oner = nc.dram_tensor("oner", [1, 128], F32R, kind="ExternalInput")
    cqn = nc.dram_tensor("cqn", [RQ, TS], F32, kind="ExternalOutput")
    ckvn = nc.dram_tensor("ckvn", [RKV, TS], F32, kind="ExternalOutput")
    kpe = nc.dram_tensor("kpe", [DR, TS], F32, kind="ExternalOutput")

    with tile.TileContext(nc) as tc:
        with tc.tile_pool(name="const", bufs=1) as cpool, \
             tc.tile_pool(name="sb", bufs=2) as sb, \
             tc.tile_pool(name="ps", bufs=1, space="PSUM") as ps:
            ones_col = cpool.tile([128, 1], F32R)
            nc.sync.dma_start(out=ones_col, in_=onec[:, :])
            ones_row = cpool.tile([1, 128], F32R)
            nc.sync.dma_start(out=ones_row, in_=oner[:, :])
            eps_t = cpool.tile([1, 1], F32)
            nc.vector.memset(eps_t, EPS)

            xt_t = cpool.tile([128, 16, TS], F32R)
            xt_r = xt[:, :].rearrange("(k p) t -> p k t", p=128)
            wqa_t = cpool.tile([128, 16, RQ], F32R)
            wkva_t = cpool.tile([128, 16, RKV + 2 * DR], F32R)
            cos_t = cpool.tile([DR, TS], F32)
            sin_t = cpool.tile([DR, TS], F32)
            for k in range(16):
                nc.sync.dma_start(out=wqa_t[:, k, :], in_=wqa[:, k, :])
                nc.sync.dma_start(out=xt_t[:, k, :], in_=xt_r[:, k, :])
            for k in range(16):
                nc.sync.dma_start(out=wkva_t[:, k, :], in_=wkva[:, k, :])
            nc.sync.dma_start(out=cos_t, in_=cosk[:, :])
            nc.sync.dma_start(out=sin_t, in_=sink[:, :])

            for path in ("q", "kv"):
                w_t = wqa_t if path == "q" else wkva_t
                out_d = cqn if path == "q" else ckvn
                nm = 4 if path == "q" else 5
                accs = []
                for m in range(nm):
                    acc = ps.tile([128, TS], F32, tag=f"mm{m}", bufs=1)
                    accs.append(acc)
                for k in range(16):
                    for m in range(nm):
                        nc.tensor.matmul(accs[m], w_t[:, k, m * 128:(m + 1) * 128],
                                         xt_t[:, k, :], start=(k == 0), stop=(k == 15))
                # variance over the 512 latent rows (m 0..3)
                var = ps.tile([1, TS], F32, tag="row", bufs=2)
                for m in range(4):
                    sq = sb.tile([128, TS], F32R, tag="sq", bufs=2)
                    nc.scalar.activation(sq, accs[m], AF.Square)
                    nc.tensor.matmul(var, ones_col, sq, start=(m == 0), stop=(m == 3))
                inv = sb.tile([1, TS], F32R, tag="inv", bufs=2)
                nc.scalar.activation(inv, var, AF.Abs_reciprocal_sqrt,
                                     scale=1.0 / 512.0, bias=eps_t[:, :])
                bc = ps.tile([128, TS], F32, tag="bc", bufs=1)
                nc.tensor.matmul(bc, ones_row, inv, start=True, stop=True)
                bcs = sb.tile([128, TS], F32, tag="bcs", bufs=1)
                nc.scalar.copy(bcs, bc)
                for m in range(4):
                    o = sb.tile([128, TS], F32, tag="no", bufs=3)
                    nc.vector.tensor_mul(o, accs[m], bcs)
                    nc.sync.dma_start(out=out_d[m * 128:(m + 1) * 128, :], in_=o)
                if path == "kv":
                    # RoPE on k_pe: rows 0:64 = pe, 64:128 = pair-swapped pe
                    pe = accs[4]
                    t0 = sb.tile([DR, TS], F32, tag="t0", bufs=1)
                    t1 = sb.tile([DR, TS], F32, tag="t1", bufs=1)
                    nc.vector.tensor_mul(t0, pe[0:DR, :], cos_t)
                    nc.vector.tensor_mul(t1, pe[DR:128, :], sin_t)
                    kp = sb.tile([DR, TS], F32, tag="kp", bufs=1)
                    nc.vector.tensor_add(kp, t0, t1)
                    nc.sync.dma_start(out=kpe[:, :], in_=kp)
    nc.compile()
    return nc


# --------------------------------------------------------------------------
# Launch 2: B-projections + q RoPE + causal attention + wo (head-sharded)
# --------------------------------------------------------------------------
def build_k2():
    nc = bacc.Bacc("TRN2", target_bir_lowering=False)
    cqn = nc.dram_tensor("cqn", [RQ, T], F32R, kind="ExternalInput")
    ckvn = nc.dram_tensor("ckvn", [RKV, T], F32R, kind="ExternalInput")
    kpe = nc.dram_tensor("kpe", [DR, T], F32R, kind="ExternalInput")
    wqb = nc.dram_tensor("wqb", [128, 4, 512], F32R, kind="ExternalInput")
    wkb = nc.dram_tensor("wkb", [128, 4, 256], F32R, kind="ExternalInput")
    wvb = nc.dram_tensor("wvb", [128, 4, 256], F32R, kind="ExternalInput")
    wop = nc.dram_tensor("wop", [128, 2, DIM], F32R, kind="ExternalInput")
    csf = nc.dram_tensor("csf", [128, S], F32, kind="ExternalInput")
    maskp = nc.dram_tensor("maskp", [128, 4, 512], F32R, kind="ExternalInput")
    onec = nc.dram_tensor("onec", [128, 1], F32R, kind="ExternalInput")
    oner = nc.dram_tensor("oner", [1, 128], F32R, kind="ExternalInput")
    out = nc.dram_tensor("out", [T, DIM], F32, kind="ExternalOutput")

    cqn_r = cqn[:, :].rearrange("(k p) t -> p k t", p=128)
    ckvn_r = ckvn[:, :].rearrange("(k p) t -> p k t", p=128)

    with tile.TileContext(nc) as tc:
        with tc.tile_pool(name="const", bufs=1) as cpool, \
             tc.tile_pool(name="perb", bufs=1) as perb, \
             tc.tile_pool(name="sb", bufs=2) as sb, \
             tc.tile_pool(name="ps", bufs=1, space="PSUM") as ps:
            ones_col = cpool.tile([128, 1], F32R)
            nc.sync.dma_start(out=ones_col, in_=onec[:, :])
            ones_row = cpool.tile([1, 128], F32R)
            nc.sync.dma_start(out=ones_row, in_=oner[:, :])
            wqb_t = cpool.tile([128, 4, 512], F32R)
            wkb_t = cpool.tile([128, 4, 256], F32R)
            wvb_t = cpool.tile([128, 4, 256], F32R)
            wop_t = cpool.tile([128, 2, DIM], F32R)
            cs_t = cpool.tile([128, S], F32)
            mask_t = cpool.tile([128, 4, 512], F32R)



            consts_loaded = False
            for b in range(B):
                qn_t = perb.tile([128, 2, S], F32R, tag="qn")
                qp_t = perb.tile([DR, 2, S], F32R, tag="qp")
                kn_t = perb.tile([128, 2, S], F32R, tag="kn")
                kp_t = perb.tile([DR, S], F32R, tag="kp")
                v_t = perb.tile([128, 16, 256], F32R, tag="v")
                o_t = perb.tile([128, 2, S], F32R, tag="o")

                # ---- B projection of one 512-token tile ----
                def proj_tt(tt):
                    nonlocal consts_loaded
                    g0 = b * S + tt * 512
                    sl = slice(tt * 512, (tt + 1) * 512)
                    cq_t = sb.tile([128, 4, 512], F32R, tag="cq", bufs=1)
                    ckv_t = sb.tile([128, 4, 512], F32R, tag="ckv", bufs=1)
                    for k in range(4):
                        if not consts_loaded:
                            nc.sync.dma_start(out=wqb_t[:, k, :], in_=wqb[:, k, :])
                        nc.sync.dma_start(out=cq_t[:, k, :], in_=cqn_r[:, k, g0:g0 + 512])
                        nc.sync.dma_start(out=ckv_t[:, k, :], in_=ckvn_r[:, k, g0:g0 + 512])
                    if not consts_loaded:
                        nc.sync.dma_start(out=cs_t, in_=csf[:, :])
                        for k in range(4):
                            nc.sync.dma_start(out=wkb_t[:, k, :], in_=wkb[:, k, :])
                            nc.sync.dma_start(out=wvb_t[:, k, :], in_=wvb[:, k, :])
                        consts_loaded = True
                    elif tt == 1 and b == 0:
                        nc.sync.dma_start(out=mask_t, in_=maskp[:, :, :])
                    elif tt == 2 and b == 0:
                        for k in range(2):
                            nc.sync.dma_start(out=wop_t[:, k, :], in_=wop[:, k, :])
                    if tt == 0:
                        nc.sync.dma_start(out=kp_t, in_=kpe[:, b * S:(b + 1) * S])

                    for m in range(4):  # h0 nope, h0 pe|swap, h1 nope, h1 pe|swap
                        acc = ps.tile([128, 512], F32, tag="mm", bufs=4)
                        for k in range(4):
                            nc.tensor.matmul(acc, wqb_t[:, k, m * 128:(m + 1) * 128],
                                             cq_t[:, k, :], start=(k == 0), stop=(k == 3))
                        h = m // 2
                        if m % 2 == 0:
                            nc.vector.tensor_copy(qn_t[:, h, sl], acc)
                        else:
                            t0 = sb.tile([DR, 512], F32, tag="t0", bufs=2)
                            t1 = sb.tile([DR, 512], F32, tag="t1", bufs=2)
                            nc.vector.tensor_mul(t0, acc[0:DR, :], cs_t[0:DR, sl])
                            nc.vector.tensor_mul(t1, acc[DR:128, :], cs_t[DR:128, sl])
                            nc.vector.tensor_add(qp_t[:, h, sl], t0, t1)
                    for m in range(2):  # k_nope per head
                        acc = ps.tile([128, 512], F32, tag="mm", bufs=4)
                        for k in range(4):
                            nc.tensor.matmul(acc, wkb_t[:, k, m * 128:(m + 1) * 128],
                                             ckv_t[:, k, :], start=(k == 0), stop=(k == 3))
                        nc.vector.tensor_copy(kn_t[:, m, sl], acc)
                    for t4 in range(4):  # v, token-major
                        acc = ps.tile([128, 256], F32, tag="mm", bufs=4)
                        for k in range(4):
                            nc.tensor.matmul(acc, ckv_t[:, k, t4 * 128:(t4 + 1) * 128],
                                             wvb_t[:, k, :], start=(k == 0), stop=(k == 3))
                        nc.vector.tensor_copy(v_t[:, tt * 4 + t4, :], acc)

                # ---- causal attention (scores transposed: [k, q]) ----
                def normalize(pend):
                    hh, lacc_p, oacc_p, qsl_p = pend
                    inv = sb.tile([1, 512], F32R, tag="inv", bufs=2)
                    with nc.allow_low_precision(reason="fp32r rounding of softmax denom"):
                        nc.vector.reciprocal(inv, lacc_p)
                    bc = ps.tile([128, 512], F32, tag="mm", bufs=4)
                    nc.tensor.matmul(bc, ones_row, inv, start=True, stop=True)
                    bcs = sb.tile([128, 512], F32, tag="bcs", bufs=2)
                    nc.vector.tensor_copy(bcs, bc)
                    nc.vector.tensor_mul(o_t[:, hh, qsl_p], oacc_p, bcs)

                wo_queue = []

                def wo_chunk(t16, ch):
                    tsl = slice(t16 * 128, (t16 + 1) * 128)
                    acc = ps.tile([128, 512], F32, tag="mm", bufs=4)
                    for hh in range(2):
                        nc.tensor.matmul(acc, o_t[:, hh, tsl],
                                         wop_t[:, hh, ch * 512:(ch + 1) * 512],
                                         start=(hh == 0), stop=(hh == 1))
                    outs = sb.tile([128, 512], F32, tag="outs", bufs=6)
                    if ch % 2 == 0:
                        nc.scalar.copy(outs, acc)
                    else:
                        nc.vector.tensor_copy(outs, acc)
                    nc.sync.dma_start(
                        out=out[b * S + t16 * 128:b * S + (t16 + 1) * 128,
                                ch * 512:(ch + 1) * 512],
                        in_=outs)

                pend_box = [None]

                def attn_qt(qt):
                    nonlocal wo_queue
                    for h in range(2):
                        qsl = slice(qt * 512, (qt + 1) * 512)
                        nkt = 4 * qt + 4
                        lacc = ps.tile([1, 512], F32, tag="row", bufs=2)
                        oacc = ps.tile([128, 512], F32, tag="pv", bufs=2)

                        lst = {"started": False, "pend": None}

                        def lacc_mm(src, off_p, w_p, last):
                            nc.tensor.matmul(lacc[:, off_p:512], ones_col,
                                             src[:, :w_p],
                                             start=(not lst["started"]), stop=last)
                            lst["started"] = True

                        def consume(prev_e):
                            et_p, off_p, w_p, kt_p = prev_e
                            last = (kt_p == nkt - 1)
                            if off_p == 0 and not last:
                                # group full-width tiles: one denominator matmul
                                # per four k-tiles (sum formed on DVE)
                                if lst["pend"] is None:
                                    lst["pend"] = et_p
                                elif lst.get("es") is None:
                                    es = sb.tile([128, 512], F32R, tag="es", bufs=2)
                                    nc.vector.tensor_add(es, lst["pend"], et_p)
                                    lst["pend"] = None
                                    lst["es"] = (es, 1)
                                else:
                                    es, n = lst["es"]
                                    nc.vector.tensor_add(es, es, et_p)
                                    if n + 1 >= 7:
                                        lacc_mm(es, 0, 512, False)
                                        lst["es"] = None
                                    else:
                                        lst["es"] = (es, n + 1)
                            else:
                                if lst["pend"] is not None:
                                    lacc_mm(lst["pend"], 0, 512, False)
                                    lst["pend"] = None
                                if lst.get("es") is not None:
                                    lacc_mm(lst["es"][0], 0, 512, False)
                                    lst["es"] = None
                                lacc_mm(et_p, off_p, w_p, last)
                            nc.tensor.matmul(oacc[:, off_p:512],
                                             v_t[:, kt_p, h * 128:(h + 1) * 128],
                                             et_p[:, :w_p],
                                             start=(kt_p == 0), stop=(kt_p == nkt - 1))

                        prev = None
                        for kt in range(nkt):
                            ksl = slice(kt * 128, (kt + 1) * 128)
                            j = kt - 4 * qt
                            # columns of this q-tile that can be unmasked:
                            off = 0 if j < 1 else (128 if j == 1 else 256)
                            w = 512 - off
                            qs2 = slice(qt * 512 + off, (qt + 1) * 512)
                            sc = ps.tile([128, 512], F32, tag="mm", bufs=4)
                            nc.tensor.matmul(sc[:, :w], kn_t[:, h, ksl],
                                             qn_t[:, h, qs2], start=True, stop=False)
                            nc.tensor.matmul(sc[:, :w], kp_t[:, ksl],
                                             qp_t[:, h, qs2], start=False, stop=True)
                            if prev is not None:
                                consume(prev)
                            et = sb.tile([128, 512], F32R, tag="exp", bufs=6)
                            nc.scalar.activation(et[:, :w], sc[:, :w], AF.Exp,
                                                 scale=SCALE)
                            if 0 <= j < 4:
                                nc.vector.tensor_mul(et[:, :w], et[:, :w],
                                                     mask_t[:, j, off:512])
                            prev = (et, off, w, kt)
                            if kt == 0:
                                if pend_box[0] is not None:
                                    normalize(pend_box[0])
                                    pend_box[0] = None
                            elif wo_queue:
                                wo_chunk(*wo_queue.pop(0))
                        consume(prev)
                        pend_box[0] = (h, lacc, oacc, qsl)
                        if h == 1:
                            wo_queue += [(t16, ch) for t16 in
                                         range(qt * 4, qt * 4 + 4) for ch in range(4)]

                # software pipeline: proj tiles feed attention one tile ahead
                proj_tt(0)
                proj_tt(1)
                attn_qt(0)
                proj_tt(2)
                attn_qt(1)
                proj_tt(3)
                attn_qt(2)
                attn_qt(3)
                if pend_box[0] is not None:
                    normalize(pend_box[0])
                    pend_box[0] = None
                for t16, ch in wo_queue:
                    wo_chunk(t16, ch)

    nc.compile()
    return nc


# --------------------------------------------------------------------------
# Host-side data prep
# --------------------------------------------------------------------------
def _pack(wT, ktiles):
    """(ktiles*128, M) -> (128, ktiles, M) with [p, k, m] = wT[k*128+p, m]."""
    K, M = wT.shape
    assert K == ktiles * 128
    return np.ascontiguousarray(wT.reshape(ktiles, 128, M).transpose(1, 0, 2))


def _swap_pairs(a, axis):
    idx = np.arange(a.shape[axis])
    idx = idx.reshape(-1, 2)[:, ::-1].reshape(-1)
    return np.take(a, idx, axis=axis)


def _prep(inputs):
    x = np.asarray(inputs["x"], dtype=np.float32)
    f = np.asarray(inputs["freqs_cis"], dtype=np.float32)
    wq_a = np.asarray(inputs["wq_a"], dtype=np.float32)
    wq_b = np.asarray(inputs["wq_b"], dtype=np.float32)
    q_norm_w = np.asarray(inputs["q_norm_w"], dtype=np.float32)
    wkv_a = np.asarray(inputs["wkv_a"], dtype=np.float32)
    kv_norm_w = np.asarray(inputs["kv_norm_w"], dtype=np.float32)
    wkv_b = np.asarray(inputs["wkv_b"], dtype=np.float32)
    wo = np.asarray(inputs["wo"], dtype=np.float32)

    xT = np.ascontiguousarray(x.reshape(T, DIM).T)  # (DIM, T)

    cos = f[:, :, 0].T  # (32, S)
    sin = f[:, :, 1].T
    cosF = np.empty((DR, S), np.float32)
    sinF = np.empty((DR, S), np.float32)
    cosF[0::2] = cos
    cosF[1::2] = cos
    sinF[0::2] = -sin
    sinF[1::2] = sin

    wqaT = wq_a.T                       # (DIM, RQ)
    wkvaT = wkv_a.T                     # (DIM, RKV+DR)
    pe = wkvaT[:, RKV:RKV + DR]
    wkva_ext = np.concatenate([wkvaT[:, :RKV], pe, _swap_pairs(pe, 1)], axis=1)
    wqa_p = _pack(wqaT, 16)
    wkva_p = _pack(wkva_ext, 16)

    k1_maps = []
    for c in range(NCORES):
        t0 = c * TS
        srange = slice(t0 % S, t0 % S + TS)
        k1_maps.append({
            "xt": np.ascontiguousarray(xT[:, t0:t0 + TS]),
            "wqa": wqa_p, "wkva": wkva_p,
            "cosk": np.ascontiguousarray(cosF[:, srange]),
            "sink": np.ascontiguousarray(sinF[:, srange]),
            "onec": np.ones((128, 1), np.float32),
            "oner": np.ones((1, 128), np.float32),
        })

    # launch-2 per-core weights
    wqbT = (wq_b * q_norm_w[None, :]).T       # (RQ, H*DQK)
    wkvbT = (wkv_b * kv_norm_w[None, :]).T    # (RKV, H*(DN+DV))
    woT = wo.T                                # (H*DV, DIM)

    masks = np.zeros((128, 4, 512), np.float32)
    kp = np.arange(128)[:, None]
    qf = np.arange(512)[None, :]
    for j in range(4):
        masks[:, j, :] = (qf >= kp + 128 * j).astype(np.float32)

    k2_maps = []
    for c in range(NCORES):
        h0, h1 = 2 * c, 2 * c + 1
        qcols = []
        for hh in (h0, h1):
            base = hh * DQK
            nope = wqbT[:, base:base + DN]
            pe_q = wqbT[:, base + DN:base + DQK]
            qcols += [nope, pe_q, _swap_pairs(pe_q, 1)]
        q_ext = np.concatenate(qcols, axis=1)             # (512, 512)
        kcols = [wkvbT[:, hh * (DN + DV):hh * (DN + DV) + DN] for hh in (h0, h1)]
        vcols = [wkvbT[:, hh * (DN + DV) + DN:(hh + 1) * (DN + DV)] for hh in (h0, h1)]
        worows = np.concatenate([woT[hh * DV:(hh + 1) * DV] for hh in (h0, h1)], axis=0)
        k2_maps.append({
            "wqb": _pack(q_ext, 4),
            "wkb": _pack(np.concatenate(kcols, axis=1), 4),
            "wvb": _pack(np.concatenate(vcols, axis=1), 4),
            "wop": _pack(worows, 2),
            "csf": np.concatenate([cosF, sinF], axis=0), "maskp": masks,
            "onec": np.ones((128, 1), np.float32),
            "oner": np.ones((1, 128), np.float32),
        })
    return k1_maps, k2_maps


def _get(name, builder):
    if name not in _CACHE:
        _CACHE[name] = builder()
    return _CACHE[name]


def _run(inputs, trace=False):
    k1_maps, k2_maps = _prep(inputs)
    nc1 = _get("k1", build_k1)
    r1 = run_bass_kernel_spmd(nc1, k1_maps, core_ids=list(range(NCORES)), trace=trace)

    cqn = np.concatenate([r1.results[c]["cqn"] for c in range(NCORES)], axis=1)
    ckvn = np.concatenate([r1.results[c]["ckvn"] for c in range(NCORES)], axis=1)
    kpe = np.concatenate([r1.results[c]["kpe"] for c in range(NCORES)], axis=1)
    for m in k2_maps:
        m["cqn"] = cqn
        m["ckvn"] = ckvn
        m["kpe"] = kpe

    nc2 = _get("k2", build_k2)
    r2 = run_bass_kernel_spmd(nc2, k2_maps, core_ids=list(range(NCORES)), trace=trace)

    acc = r2.results[0]["out"].astype(np.float32)
    for c in range(1, NCORES):
        acc = acc + r2.results[c]["out"]
    return acc.reshape(B, S, DIM), (r1, r2)


def kernel(**inputs) -> np.ndarray:
    out, _ = _run(inputs)
    return out



# revision 2
# speedup vs baseline: 1.0622x; 1.0622x over previous
"""MLA-style attention (nn_Attention_15496242004691) on 8 trn2 NeuronCores.

Strategy:
  Launch 1 (token-sharded, 512 tokens/core): cq = x@wq_a.T, ckv = x@wkv_a.T,
    RMSNorm of both (norm weights folded into the B projections on host),
    RoPE of k_pe (pair-swap folded into an extended wkv_a on host).
  Launch 2 (head-sharded, 2 heads/core): B projections (+q RoPE), causal
    attention with transposed scores (softmax sums via ones-matmul),
    output projection; host sums the 8 partial outputs.

All tensors are bf16 except PSUM accumulation (fp32), the softmax
denominators (fp32 PSUM via ones-matmuls over zero-padded exp tiles) and the
final output (fp32 partials summed on host). Activations are kept
feature-on-partition so no on-chip transposes occur.
"""

import numpy as np
import ml_dtypes

import concourse.bass as bass
import concourse.mybir as mybir
import concourse.tile as tile
from concourse import bacc
from concourse.bass_utils import run_bass_kernel_spmd

F32 = mybir.dt.float32
F32R = mybir.dt.float32r
BF16 = mybir.dt.bfloat16
NPBF = ml_dtypes.bfloat16
AF = mybir.ActivationFunctionType
OP = mybir.AluOpType

B, S, DIM, H = 2, 2048, 2048, 16
NCORES = 8
HPC = H // NCORES  # heads per core = 2
RQ = RKV = 512
DN, DR, DV, DQK = 128, 64, 128, 192
EPS = 1e-6
SCALE = DQK ** -0.5
T = B * S          # 4096 tokens
TS = T // NCORES   # 512 tokens per core in launch 1
ES_GROUP = 6       # exp tiles summed on DVE per softmax-denominator matmul

_CACHE = {}


# --------------------------------------------------------------------------
# Launch 1: A-projections + RMSNorm + k_pe RoPE (token-sharded)
# --------------------------------------------------------------------------
def build_k1():
    nc = bacc.Bacc("TRN2", target_bir_lowering=False)
    xt = nc.dram_tensor("xt", [DIM, TS], BF16, kind="ExternalInput")
    wqa = nc.dram_tensor("wqa", [128, 16, RQ], BF16, kind="ExternalInput")
    wkva = nc.dram_tensor("wkva", [128, 16, RKV + 2 * DR], BF16, kind="ExternalInput")
    cosk = nc.dram_tensor("cosk", [DR, TS], BF16, kind="ExternalInput")
    sink = nc.dram_tensor("sink", [DR, TS], BF16, kind="ExternalInput")
    onec = nc.dram_tensor("onec", [128, 1], BF16, kind="ExternalInput")
    oner = nc.dram_tensor("oner", [1, 128], F32R, kind="ExternalInput")
    cqn = nc.dram_tensor("cqn", [RQ, TS], BF16, kind="ExternalOutput")
    ckvn = nc.dram_tensor("ckvn", [RKV, TS], BF16, kind="ExternalOutput")
    kpe = nc.dram_tensor("kpe", [DR, TS], BF16, kind="ExternalOutput")

    with tile.TileContext(nc) as tc:
        with tc.tile_pool(name="const", bufs=1) as cpool, \
             tc.tile_pool(name="sb", bufs=2) as sb, \
             tc.tile_pool(name="ps", bufs=1, space="PSUM") as ps:
            ones_col = cpool.tile([128, 1], BF16)
            nc.sync.dma_start(out=ones_col, in_=onec[:, :])
            ones_row = cpool.tile([1, 128], F32R)
            nc.sync.dma_start(out=ones_row, in_=oner[:, :])
            eps_t = cpool.tile([1, 1], F32)
            nc.vector.memset(eps_t, EPS)

            xt_t = cpool.tile([128, 16, TS], BF16)
            xt_r = xt[:, :].rearrange("(k p) t -> p k t", p=128)
            wqa_t = cpool.tile([128, 16, RQ], BF16)
            wkva_t = cpool.tile([128, 16, RKV + 2 * DR], BF16)
            cos_t = cpool.tile([DR, TS], BF16)
            sin_t = cpool.tile([DR, TS], BF16)
            for k in range(16):
                nc.sync.dma_start(out=wqa_t[:, k, :], in_=wqa[:, k, :])
                nc.sync.dma_start(out=xt_t[:, k, :], in_=xt_r[:, k, :])
            for k in range(16):
                nc.sync.dma_start(out=wkva_t[:, k, :], in_=wkva[:, k, :])
            nc.sync.dma_start(out=cos_t, in_=cosk[:, :])
            nc.sync.dma_start(out=sin_t, in_=sink[:, :])

            for path in ("q", "kv"):
                w_t = wqa_t if path == "q" else wkva_t
                out_d = cqn if path == "q" else ckvn
                nm = 4 if path == "q" else 5
                accs = []
                for m in range(nm):
                    acc = ps.tile([128, TS], F32, tag=f"mm{m}", bufs=1)
                    accs.append(acc)
                for k in range(16):
                    for m in range(nm):
                        nc.tensor.matmul(accs[m], w_t[:, k, m * 128:(m + 1) * 128],
                                         xt_t[:, k, :], start=(k == 0), stop=(k == 15))
                # variance over the 512 latent rows (m 0..3)
                var = ps.tile([1, TS], F32, tag="row", bufs=2)
                for m in range(4):
                    sq = sb.tile([128, TS], BF16, tag="sq", bufs=2)
                    nc.scalar.activation(sq, accs[m], AF.Square)
                    nc.tensor.matmul(var, ones_col, sq, start=(m == 0), stop=(m == 3))
                inv = sb.tile([1, TS], F32R, tag="inv", bufs=2)
                nc.scalar.activation(inv, var, AF.Abs_reciprocal_sqrt,
                                     scale=1.0 / 512.0, bias=eps_t[:, :])
                bc = ps.tile([128, TS], F32, tag="bc", bufs=1)
                nc.tensor.matmul(bc, ones_row, inv, start=True, stop=True)
                bcs = sb.tile([128, TS], F32, tag="bcs", bufs=1)
                nc.scalar.copy(bcs, bc)
                for m in range(4):
                    o = sb.tile([128, TS], BF16, tag="no", bufs=3)
                    nc.vector.tensor_mul(o, accs[m], bcs)
                    nc.sync.dma_start(out=out_d[m * 128:(m + 1) * 128, :], in_=o)
                if path == "kv":
                    # RoPE on k_pe: rows 0:64 = pe, 64:128 = pair-swapped pe
                    pe = accs[4]
                    t0 = sb.tile([DR, TS], BF16, tag="t0", bufs=1)
                    t1 = sb.tile([DR, TS], BF16, tag="t1", bufs=1)
                    nc.vector.tensor_mul(t0, pe[0:DR, :], cos_t)
                    nc.vector.tensor_mul(t1, pe[DR:128, :], sin_t)
                    kp = sb.tile([DR, TS], BF16, tag="kp", bufs=1)
                    nc.vector.tensor_add(kp, t0, t1)
                    nc.sync.dma_start(out=kpe[:, :], in_=kp)
    nc.compile()
    return nc


# --------------------------------------------------------------------------
# Launch 2: B-projections + q RoPE + causal attention + wo (head-sharded)
# --------------------------------------------------------------------------
def build_k2():
    nc = bacc.Bacc("TRN2", target_bir_lowering=False)
    cqn = nc.dram_tensor("cqn", [RQ, T], BF16, kind="ExternalInput")
    ckvn = nc.dram_tensor("ckvn", [RKV, T], BF16, kind="ExternalInput")
    kpe = nc.dram_tensor("kpe", [DR, T], BF16, kind="ExternalInput")
    wqb = nc.dram_tensor("wqb", [128, 4, 512], BF16, kind="ExternalInput")
    wkb = nc.dram_tensor("wkb", [128, 4, 256], BF16, kind="ExternalInput")
    wvb = nc.dram_tensor("wvb", [128, 4, 256], BF16, kind="ExternalInput")
    wop = nc.dram_tensor("wop", [128, 2, DIM], BF16, kind="ExternalInput")
    csf = nc.dram_tensor("csf", [128, S], BF16, kind="ExternalInput")
    trim = nc.dram_tensor("trim", [128, 128], BF16, kind="ExternalInput")
    onec = nc.dram_tensor("onec", [128, 1], BF16, kind="ExternalInput")
    oner = nc.dram_tensor("oner", [1, 128], F32R, kind="ExternalInput")
    out = nc.dram_tensor("out", [T, DIM], F32, kind="ExternalOutput")

    cqn_r = cqn[:, :].rearrange("(k p) t -> p k t", p=128)
    ckvn_r = ckvn[:, :].rearrange("(k p) t -> p k t", p=128)

    with tile.TileContext(nc) as tc:
        with tc.tile_pool(name="const", bufs=1) as cpool, \
             tc.tile_pool(name="perb", bufs=1) as perb, \
             tc.tile_pool(name="sb", bufs=2) as sb, \
             tc.tile_pool(name="ps", bufs=1, space="PSUM") as ps:
            ones_col = cpool.tile([128, 1], BF16)
            nc.sync.dma_start(out=ones_col, in_=onec[:, :])
            ones_row = cpool.tile([1, 128], F32R)
            nc.sync.dma_start(out=ones_row, in_=oner[:, :])
            wqb_t = cpool.tile([128, 4, 512], BF16)
            wkb_t = cpool.tile([128, 4, 256], BF16)
            wvb_t = cpool.tile([128, 4, 256], BF16)
            wop_t = cpool.tile([128, 2, DIM], BF16)
            cs_t = cpool.tile([128, S], BF16)
            trim_t = cpool.tile([128, 128], BF16)

            consts_loaded = False
            for b in range(B):
                qn_t = perb.tile([128, 2, S], BF16, tag="qn")
                qp_t = perb.tile([DR, 2, S], BF16, tag="qp")
                kn_t = perb.tile([128, 2, S], BF16, tag="kn")
                kp_t = perb.tile([DR, S], BF16, tag="kp")
                v_t = perb.tile([128, 16, 256], BF16, tag="v")
                o_t = perb.tile([128, 2, S], BF16, tag="o")

                # ---- B projection of one 512-token tile ----
                def proj_tt(tt):
                    nonlocal consts_loaded
                    g0 = b * S + tt * 512
                    sl = slice(tt * 512, (tt + 1) * 512)
                    cq_t = sb.tile([128, 4, 512], BF16, tag="cq", bufs=1)
                    ckv_t = sb.tile([128, 4, 512], BF16, tag="ckv", bufs=1)
                    for k in range(4):
                        if not consts_loaded:
                            nc.sync.dma_start(out=wqb_t[:, k, :], in_=wqb[:, k, :])
                        nc.sync.dma_start(out=cq_t[:, k, :], in_=cqn_r[:, k, g0:g0 + 512])
                        nc.sync.dma_start(out=ckv_t[:, k, :], in_=ckvn_r[:, k, g0:g0 + 512])
                    if not consts_loaded:
                        nc.sync.dma_start(out=cs_t, in_=csf[:, :])
                        for k in range(4):
                            nc.sync.dma_start(out=wkb_t[:, k, :], in_=wkb[:, k, :])
                            nc.sync.dma_start(out=wvb_t[:, k, :], in_=wvb[:, k, :])
                        consts_loaded = True
                    elif tt == 1 and b == 0:
                        nc.sync.dma_start(out=trim_t, in_=trim[:, :])
                    elif tt == 2 and b == 0:
                        for k in range(2):
                            nc.sync.dma_start(out=wop_t[:, k, :], in_=wop[:, k, :])
                    if tt == 0:
                        nc.sync.dma_start(out=kp_t, in_=kpe[:, b * S:(b + 1) * S])

                    for m in range(4):  # h0 nope, h0 pe|swap, h1 nope, h1 pe|swap
                        acc = ps.tile([128, 512], F32, tag="mm", bufs=4)
                        for k in range(4):
                            nc.tensor.matmul(acc, wqb_t[:, k, m * 128:(m + 1) * 128],
                                             cq_t[:, k, :], start=(k == 0), stop=(k == 3))
                        h = m // 2
                        if m % 2 == 0:
                            nc.vector.tensor_copy(qn_t[:, h, sl], acc)
                        else:
                            pe_s = sb.tile([128, 512], BF16, tag="pes", bufs=2)
                            nc.scalar.copy(pe_s, acc)
                            t0 = sb.tile([DR, 512], BF16, tag="t0", bufs=2)
                            t1 = sb.tile([DR, 512], BF16, tag="t1", bufs=2)
                            nc.vector.tensor_mul(t0, pe_s[0:DR, :], cs_t[0:DR, sl])
                            nc.vector.tensor_mul(t1, pe_s[DR:128, :], cs_t[DR:128, sl])
                            nc.vector.tensor_add(qp_t[:, h, sl], t0, t1)
                    for m in range(2):  # k_nope per head
                        acc = ps.tile([128, 512], F32, tag="mm", bufs=4)
                        for k in range(4):
                            nc.tensor.matmul(acc, wkb_t[:, k, m * 128:(m + 1) * 128],
                                             ckv_t[:, k, :], start=(k == 0), stop=(k == 3))
                        nc.scalar.copy(kn_t[:, m, sl], acc)
                    for t4 in range(4):  # v, token-major
                        acc = ps.tile([128, 256], F32, tag="mm", bufs=4)
                        for k in range(4):
                            nc.tensor.matmul(acc, ckv_t[:, k, t4 * 128:(t4 + 1) * 128],
                                             wvb_t[:, k, :], start=(k == 0), stop=(k == 3))
                        nc.vector.tensor_copy(v_t[:, tt * 4 + t4, :], acc)

                # ---- causal attention (scores transposed: [k, q]) ----
                def normalize(pend):
                    hh, lacc_p, oacc_p, qsl_p = pend
                    inv = sb.tile([1, 512], F32R, tag="inv", bufs=2)
                    with nc.allow_low_precision(reason="fp32r rounding of softmax denom"):
                        nc.vector.reciprocal(inv, lacc_p)
                    bc = ps.tile([128, 512], F32, tag="mm", bufs=4)
                    nc.tensor.matmul(bc, ones_row, inv, start=True, stop=True)
                    bcs = sb.tile([128, 512], F32, tag="bcs", bufs=2)
                    nc.vector.tensor_copy(bcs, bc)
                    nc.vector.tensor_mul(o_t[:, hh, qsl_p], oacc_p, bcs)

                wo_queue = []

                def wo_chunk(t16, ch):
                    tsl = slice(t16 * 128, (t16 + 1) * 128)
                    acc = ps.tile([128, 512], F32, tag="mm", bufs=4)
                    for hh in range(2):
                        nc.tensor.matmul(acc, o_t[:, hh, tsl],
                                         wop_t[:, hh, ch * 512:(ch + 1) * 512],
                                         start=(hh == 0), stop=(hh == 1))
                    outs = sb.tile([128, 512], F32, tag="outs", bufs=6)
                    if ch % 2 == 0:
                        nc.scalar.copy(outs, acc)
                    else:
                        nc.vector.tensor_copy(outs, acc)
                    nc.sync.dma_start(
                        out=out[b * S + t16 * 128:b * S + (t16 + 1) * 128,
                                ch * 512:(ch + 1) * 512],
                        in_=outs)

                pend_box = [None]

                def attn_qt(qt):
                    nonlocal wo_queue
                    for h in range(2):
                        qsl = slice(qt * 512, (qt + 1) * 512)
                        nkt = 4 * qt + 4
                        lacc = ps.tile([1, 512], F32, tag="row", bufs=2)
                        oacc = ps.tile([128, 512], F32, tag="pv", bufs=2)

                        # softmax-denominator batching: exp tiles are zero-
                        # padded below their causal offset, so full-width DVE
                        # adds accumulate groups and one ones-matmul per group
                        # lands the column sums in PSUM (fp32, exact).
                        lst = {"pend": None, "es": None, "cnt": 0, "started": False}

                        def lacc_mm(src, last):
                            nc.tensor.matmul(lacc, ones_col, src,
                                             start=(not lst["started"]), stop=last)
                            lst["started"] = True

                        def consume(prev_e):
                            et_p, off_p, kt_p = prev_e
                            last = (kt_p == nkt - 1)
                            nc.tensor.matmul(oacc[:, off_p:512],
                                             v_t[:, kt_p, h * 128:(h + 1) * 128],
                                             et_p[:, off_p:512],
                                             start=(kt_p == 0), stop=last)
                            if lst["pend"] is None and lst["es"] is None:
                                lst["pend"] = et_p
                                lst["cnt"] = 1
                            elif lst["es"] is None:
                                es = sb.tile([128, 512], BF16, tag="es", bufs=2)
                                nc.vector.tensor_add(es, lst["pend"], et_p)
                                lst["pend"] = None
                                lst["es"] = es
                                lst["cnt"] = 2
                            else:
                                nc.vector.tensor_add(lst["es"], lst["es"], et_p)
                                lst["cnt"] += 1
                            if last or lst["cnt"] >= ES_GROUP:
                                src = lst["es"] if lst["es"] is not None else lst["pend"]
                                lacc_mm(src, last)
                                lst["pend"] = None
                                lst["es"] = None
                                lst["cnt"] = 0

                        prev = None
                        for kt in range(nkt):
                            ksl = slice(kt * 128, (kt + 1) * 128)
                            j = kt - 4 * qt
                            # columns of this q-tile that can be unmasked:
                            off = 128 * j if j > 0 else 0
                            w = 512 - off
                            qs2 = slice(qt * 512 + off, (qt + 1) * 512)
                            sc = ps.tile([128, 512], F32, tag="mm", bufs=4)
                            nc.tensor.matmul(sc[:, :w], kn_t[:, h, ksl],
                                             qn_t[:, h, qs2], start=True, stop=False)
                            nc.tensor.matmul(sc[:, :w], kp_t[:, ksl],
                                             qp_t[:, h, qs2], start=False, stop=True)
                            if prev is not None:
                                consume(prev)
                            et = sb.tile([128, 512], BF16, tag="exp", bufs=6)
                            if j > 0:
                                nc.gpsimd.memset(et[:, :off], 0.0)
                            nc.scalar.activation(et[:, off:512], sc[:, :w], AF.Exp,
                                                 scale=SCALE)
                            if 0 <= j < 4:
                                nc.vector.tensor_mul(et[:, off:off + 128],
                                                     et[:, off:off + 128], trim_t)
                            prev = (et, off, kt)
                            if kt == 0:
                                if pend_box[0] is not None:
                                    normalize(pend_box[0])
                                    pend_box[0] = None
                            elif wo_queue:
                                wo_chunk(*wo_queue.pop(0))
                        consume(prev)
                        pend_box[0] = (h, lacc, oacc, qsl)
                        if h == 1:
                            wo_queue += [(t16, ch) for t16 in
                                         range(qt * 4, qt * 4 + 4) for ch in range(4)]

                # software pipeline: proj tiles feed attention one tile ahead
                proj_tt(0)
                proj_tt(1)
                attn_qt(0)
                proj_tt(2)
                attn_qt(1)
                proj_tt(3)
                attn_qt(2)
                attn_qt(3)
                if pend_box[0] is not None:
                    normalize(pend_box[0])
                    pend_box[0] = None
                for t16, ch in wo_queue:
                    wo_chunk(t16, ch)

    nc.compile()
    return nc


# --------------------------------------------------------------------------
# Host-side data prep
# --------------------------------------------------------------------------
def _pack(wT, ktiles):
    """(ktiles*128, M) -> (128, ktiles, M) with [p, k, m] = wT[k*128+p, m]."""
    K, M = wT.shape
    assert K == ktiles * 128
    return np.ascontiguousarray(
        wT.reshape(ktiles, 128, M).transpose(1, 0, 2)).astype(NPBF)


def _swap_pairs(a, axis):
    idx = np.arange(a.shape[axis])
    idx = idx.reshape(-1, 2)[:, ::-1].reshape(-1)
    return np.take(a, idx, axis=axis)


def _prep(inputs):
    x = np.asarray(inputs["x"], dtype=np.float32)
    f = np.asarray(inputs["freqs_cis"], dtype=np.float32)
    wq_a = np.asarray(inputs["wq_a"], dtype=np.float32)
    wq_b = np.asarray(inputs["wq_b"], dtype=np.float32)
    q_norm_w = np.asarray(inputs["q_norm_w"], dtype=np.float32)
    wkv_a = np.asarray(inputs["wkv_a"], dtype=np.float32)
    kv_norm_w = np.asarray(inputs["kv_norm_w"], dtype=np.float32)
    wkv_b = np.asarray(inputs["wkv_b"], dtype=np.float32)
    wo = np.asarray(inputs["wo"], dtype=np.float32)

    xT = np.ascontiguousarray(x.reshape(T, DIM).T).astype(NPBF)  # (DIM, T)

    cos = f[:, :, 0].T  # (32, S)
    sin = f[:, :, 1].T
    cosF = np.empty((DR, S), np.float32)
    sinF = np.empty((DR, S), np.float32)
    cosF[0::2] = cos
    cosF[1::2] = cos
    sinF[0::2] = -sin
    sinF[1::2] = sin

    wqaT = wq_a.T                       # (DIM, RQ)
    wkvaT = wkv_a.T                     # (DIM, RKV+DR)
    pe = wkvaT[:, RKV:RKV + DR]
    wkva_ext = np.concatenate([wkvaT[:, :RKV], pe, _swap_pairs(pe, 1)], axis=1)
    wqa_p = _pack(wqaT, 16)
    wkva_p = _pack(wkva_ext, 16)

    onec = np.ones((128, 1), NPBF)
    oner = np.ones((1, 128), np.float32)
    cosB = cosF.astype(NPBF)
    sinB = sinF.astype(NPBF)

    k1_maps = []
    for c in range(NCORES):
        t0 = c * TS
        srange = slice(t0 % S, t0 % S + TS)
        k1_maps.append({
            "xt": np.ascontiguousarray(xT[:, t0:t0 + TS]),
            "wqa": wqa_p, "wkva": wkva_p,
            "cosk": np.ascontiguousarray(cosB[:, srange]),
            "sink": np.ascontiguousarray(sinB[:, srange]),
            "onec": onec,
            "oner": oner,
        })

    # launch-2 per-core weights
    wqbT = (wq_b * q_norm_w[None, :]).T       # (RQ, H*DQK)
    wkvbT = (wkv_b * kv_norm_w[None, :]).T    # (RKV, H*(DN+DV))
    woT = wo.T                                # (H*DV, DIM)

    # strict-lower-triangle keep-mask for the 128x128 diagonal sub-block
    kk = np.arange(128)[:, None]
    qq = np.arange(128)[None, :]
    trim = (qq >= kk).astype(NPBF)

    csf = np.concatenate([cosF, sinF], axis=0).astype(NPBF)

    k2_maps = []
    for c in range(NCORES):
        h0, h1 = 2 * c, 2 * c + 1
        qcols = []
        for hh in (h0, h1):
            base = hh * DQK
            nope = wqbT[:, base:base + DN]
            pe_q = wqbT[:, base + DN:base + DQK]
            qcols += [nope, pe_q, _swap_pairs(pe_q, 1)]
        q_ext = np.concatenate(qcols, axis=1)             # (512, 512)
        kcols = [wkvbT[:, hh * (DN + DV):hh * (DN + DV) + DN] for hh in (h0, h1)]
        vcols = [wkvbT[:, hh * (DN + DV) + DN:(hh + 1) * (DN + DV)] for hh in (h0, h1)]
        worows = np.concatenate([woT[hh * DV:(hh + 1) * DV] for hh in (h0, h1)], axis=0)
        k2_maps.append({
            "wqb": _pack(q_ext, 4),
            "wkb": _pack(np.concatenate(kcols, axis=1), 4),
            "wvb": _pack(np.concatenate(vcols, axis=1), 4),
            "wop": _pack(worows, 2),
            "csf": csf, "trim": trim,
            "onec": onec,
            "oner": oner,
        })
    return k1_maps, k2_maps


def _get(name, builder):
    if name not in _CACHE:
        _CACHE[name] = builder()
    return _CACHE[name]


def _run(inputs, trace=False):
    k1_maps, k2_maps = _prep(inputs)
    nc1 = _get("k1", build_k1)
    r1 = run_bass_kernel_spmd(nc1, k1_maps, core_ids=list(range(NCORES)), trace=trace)

    cqn = np.concatenate([r1.results[c]["cqn"] for c in range(NCORES)], axis=1)
    ckvn = np.concatenate([r1.results[c]["ckvn"] for c in range(NCORES)], axis=1)
    kpe = np.concatenate([r1.results[c]["kpe"] for c in range(NCORES)], axis=1)
    for m in k2_maps:
        m["cqn"] = cqn
        m["ckvn"] = ckvn
        m["kpe"] = kpe

    nc2 = _get("k2", build_k2)
    r2 = run_bass_kernel_spmd(nc2, k2_maps, core_ids=list(range(NCORES)), trace=trace)

    acc = r2.results[0]["out"].astype(np.float32)
    for c in range(1, NCORES):
        acc = acc + r2.results[c]["out"]
    return acc.reshape(B, S, DIM), (r1, r2)


def kernel(**inputs) -> np.ndarray:
    out, _ = _run(inputs)
    return out


# revision 10
# speedup vs baseline: 1.1761x; 1.1072x over previous
"""MLA-style attention (nn_Attention_15496242004691) on 8 trn2 NeuronCores.

Strategy:
  Launch 1 (token-sharded, 512 tokens/core): cq = x@wq_a.T, ckv = x@wkv_a.T,
    RMSNorm of both (norm weights folded into the B projections on host),
    RoPE of k_pe (pair-swap folded into an extended wkv_a on host).
  Launch 2 (head-sharded, 2 heads/core): B projections (+q RoPE), causal
    attention with transposed scores (softmax column sums via GPSIMD
    partition_all_reduce over batched exp tiles), output projection; host
    sums the 8 partial outputs.

All tensors are bf16 except PSUM accumulation (fp32), the softmax
denominators (fp32) and the final output (fp32 partials summed on host).
Activations are kept feature-on-partition so no on-chip transposes occur.
DMAs are batched into few large transfers: descriptor generation (HWDGE) is
a serial ~625ns/DMA resource that a naive per-tile DMA schedule saturates.
"""

import numpy as np
import ml_dtypes

import concourse.bass as bass
import concourse.bass_isa as bass_isa
import concourse.mybir as mybir
import concourse.tile as tile
from concourse import bacc, library_config
from concourse.bass_utils import run_bass_kernel_spmd

F32 = mybir.dt.float32
F32R = mybir.dt.float32r
BF16 = mybir.dt.bfloat16
NPBF = ml_dtypes.bfloat16
AF = mybir.ActivationFunctionType
OP = mybir.AluOpType
RADD = bass_isa.ReduceOp.add

B, S, DIM, H = 2, 2048, 2048, 16
NCORES = 8
HPC = H // NCORES  # heads per core = 2
RQ = RKV = 512
DN, DR, DV, DQK = 128, 64, 128, 192
EPS = 1e-6
SCALE = DQK ** -0.5
T = B * S          # 4096 tokens
TS = T // NCORES   # 512 tokens per core in launch 1
ES_GROUP = 6       # exp tiles summed on DVE per partition_all_reduce

_CACHE = {}


# --------------------------------------------------------------------------
# Launch 1: A-projections + RMSNorm + k_pe RoPE (token-sharded)
# --------------------------------------------------------------------------
def build_k1():
    nc = bacc.Bacc("TRN2", target_bir_lowering=False)
    xt = nc.dram_tensor("xt", [DIM, TS], BF16, kind="ExternalInput")
    wqa = nc.dram_tensor("wqa", [128, 16, RQ], BF16, kind="ExternalInput")
    wkva = nc.dram_tensor("wkva", [128, 16, RKV + 2 * DR], BF16, kind="ExternalInput")
    csk = nc.dram_tensor("csk", [128, TS], BF16, kind="ExternalInput")
    cqn = nc.dram_tensor("cqn", [RQ, TS], BF16, kind="ExternalOutput")
    ckvn = nc.dram_tensor("ckvn", [RKV, TS], BF16, kind="ExternalOutput")
    kpe = nc.dram_tensor("kpe", [DR, TS], BF16, kind="ExternalOutput")

    cqn_r = cqn[:, :].rearrange("(m p) t -> p m t", p=128)
    ckvn_r = ckvn[:, :].rearrange("(m p) t -> p m t", p=128)

    with tile.TileContext(nc) as tc:
        with tc.tile_pool(name="const", bufs=1) as cpool, \
             tc.tile_pool(name="sb", bufs=2) as sb, \
             tc.tile_pool(name="ps", bufs=1, space="PSUM") as ps:
            nc.gpsimd.load_library(library_config.attn)
            eps_t = cpool.tile([128, 1], F32)
            nc.vector.memset(eps_t, EPS)

            xt_t = cpool.tile([128, 16, TS], BF16)
            xt_r = xt[:, :].rearrange("(k p) t -> p k t", p=128)
            wqa_t = cpool.tile([128, 16, RQ], BF16)
            wkva_t = cpool.tile([128, 16, RKV + 2 * DR], BF16)
            cs_t = cpool.tile([128, TS], BF16)
            # batched loads, split so the first matmuls can start early
            nc.sync.dma_start(out=wqa_t[:, 0:4, :], in_=wqa[:, 0:4, :])
            nc.sync.dma_start(out=xt_t[:, 0:4, :], in_=xt_r[:, 0:4, :])
            nc.sync.dma_start(out=wqa_t[:, 4:16, :], in_=wqa[:, 4:16, :])
            nc.sync.dma_start(out=xt_t[:, 4:16, :], in_=xt_r[:, 4:16, :])
            nc.sync.dma_start(out=wkva_t, in_=wkva[:, :, :])
            nc.sync.dma_start(out=cs_t, in_=csk[:, :])

            for path in ("q", "kv"):
                w_t = wqa_t if path == "q" else wkva_t
                out_r = cqn_r if path == "q" else ckvn_r
                nm = 4 if path == "q" else 5
                cqu = sb.tile([128, 4, TS], BF16, tag=f"cqu{path}", bufs=1)
                o_all = sb.tile([128, 4, TS], BF16, tag=f"oall{path}", bufs=1)
                ss = sb.tile([128, TS], BF16, tag="ss", bufs=2)
                pe_s = None
                for m in range(nm):
                    acc = ps.tile([128, TS], F32, tag="mm", bufs=3)
                    for k in range(16):
                        nc.tensor.matmul(acc, w_t[:, k, m * 128:(m + 1) * 128],
                                         xt_t[:, k, :], start=(k == 0), stop=(k == 15))
                    if m < 4:
                        sq = sb.tile([128, TS], BF16, tag="sq", bufs=2)
                        nc.scalar.activation(sq, acc, AF.Square)
                        nc.vector.tensor_copy(cqu[:, m, :], acc)
                        if m == 0:
                            first_sq = sq
                        elif m == 1:
                            nc.vector.tensor_add(ss, first_sq, sq)
                        else:
                            nc.vector.tensor_add(ss, ss, sq)
                    else:
                        pe_s = sb.tile([128, TS], BF16, tag="pes", bufs=1)
                        nc.scalar.copy(pe_s, acc)
                # rsqrt(mean-square + eps), broadcast across partitions
                var_bc = sb.tile([128, TS], F32, tag="var", bufs=2)
                nc.gpsimd.partition_all_reduce(var_bc, ss, channels=128,
                                               reduce_op=RADD)
                bcs = sb.tile([128, TS], BF16, tag="bcs", bufs=2)
                nc.scalar.activation(bcs, var_bc, AF.Abs_reciprocal_sqrt,
                                     scale=1.0 / 512.0, bias=eps_t[:, :])
                for m in range(4):
                    nc.vector.tensor_mul(o_all[:, m, :], cqu[:, m, :], bcs)
                nc.sync.dma_start(out=out_r[:, :, :], in_=o_all)
                if path == "kv":
                    # RoPE on k_pe: rows 0:64 = pe, 64:128 = pair-swapped pe
                    t0 = sb.tile([DR, TS], BF16, tag="t0", bufs=1)
                    t1 = sb.tile([DR, TS], BF16, tag="t1", bufs=1)
                    nc.vector.tensor_mul(t0, pe_s[0:DR, :], cs_t[0:DR, :])
                    nc.vector.tensor_mul(t1, pe_s[DR:128, :], cs_t[DR:128, :])
                    kp = sb.tile([DR, TS], BF16, tag="kp", bufs=1)
                    nc.vector.tensor_add(kp, t0, t1)
                    nc.sync.dma_start(out=kpe[:, :], in_=kp)
    nc.compile()
    return nc


# --------------------------------------------------------------------------
# Launch 2: B-projections + q RoPE + causal attention + wo (head-sharded)
# --------------------------------------------------------------------------
def build_k2():
    nc = bacc.Bacc("TRN2", target_bir_lowering=False)
    cqn = nc.dram_tensor("cqn", [RQ, T], BF16, kind="ExternalInput")
    ckvn = nc.dram_tensor("ckvn", [RKV, T], BF16, kind="ExternalInput")
    kpe = nc.dram_tensor("kpe", [DR, T], BF16, kind="ExternalInput")
    wqb = nc.dram_tensor("wqb", [128, 4, 512], BF16, kind="ExternalInput")
    wkb = nc.dram_tensor("wkb", [128, 4, 256], BF16, kind="ExternalInput")
    wvb = nc.dram_tensor("wvb", [128, 4, 256], BF16, kind="ExternalInput")
    wop = nc.dram_tensor("wop", [128, 2, DIM], BF16, kind="ExternalInput")
    csf = nc.dram_tensor("csf", [128, S], BF16, kind="ExternalInput")
    trim = nc.dram_tensor("trim", [128, 128], BF16, kind="ExternalInput")
    out = nc.dram_tensor("out", [T, DIM], F32, kind="ExternalOutput")

    cqn_r = cqn[:, :].rearrange("(k p) t -> p k t", p=128)
    ckvn_r = ckvn[:, :].rearrange("(k p) t -> p k t", p=128)

    with tile.TileContext(nc) as tc:
        with tc.tile_pool(name="const", bufs=1) as cpool, \
             tc.tile_pool(name="perb", bufs=1) as perb, \
             tc.tile_pool(name="sb", bufs=2) as sb, \
             tc.tile_pool(name="ps", bufs=1, space="PSUM") as ps:
            nc.gpsimd.load_library(library_config.attn)
            wqb_t = cpool.tile([128, 4, 512], BF16)
            wkb_t = cpool.tile([128, 4, 256], BF16)
            wvb_t = cpool.tile([128, 4, 256], BF16)
            wop_t = cpool.tile([128, 2, DIM], BF16)
            cs_t = cpool.tile([128, S], BF16)
            trim_t = cpool.tile([128, 128], BF16)

            consts_loaded = False
            for b in range(B):
                qn_t = perb.tile([128, 2, S], BF16, tag="qn")
                qp_t = perb.tile([DR, 2, S], BF16, tag="qp")
                kn_t = perb.tile([128, 2, S], BF16, tag="kn")
                kp_t = perb.tile([DR, S], BF16, tag="kp")
                v_t = perb.tile([128, 16, 256], BF16, tag="v")
                o_t = perb.tile([128, 2, S], BF16, tag="o")

                # ---- B projection of one 512-token tile ----
                def proj_tt(tt):
                    nonlocal consts_loaded
                    g0 = b * S + tt * 512
                    sl = slice(tt * 512, (tt + 1) * 512)
                    cq_t = sb.tile([128, 4, 512], BF16, tag="cq", bufs=2)
                    ckv_t = sb.tile([128, 4, 512], BF16, tag="ckv", bufs=2)
                    if not consts_loaded:
                        nc.sync.dma_start(out=wqb_t, in_=wqb[:, :, :])
                    nc.sync.dma_start(out=cq_t, in_=cqn_r[:, :, g0:g0 + 512])
                    nc.sync.dma_start(out=ckv_t, in_=ckvn_r[:, :, g0:g0 + 512])
                    if not consts_loaded:
                        nc.sync.dma_start(out=cs_t, in_=csf[:, :])
                        nc.sync.dma_start(out=wkb_t, in_=wkb[:, :, :])
                        nc.sync.dma_start(out=wvb_t, in_=wvb[:, :, :])
                        consts_loaded = True
                    elif tt == 1 and b == 0:
                        nc.sync.dma_start(out=trim_t, in_=trim[:, :])
                    elif tt == 2 and b == 0:
                        nc.sync.dma_start(out=wop_t, in_=wop[:, :, :])
                    if tt == 0:
                        nc.sync.dma_start(out=kp_t, in_=kpe[:, b * S:(b + 1) * S])

                    for m in range(4):  # h0 nope, h0 pe|swap, h1 nope, h1 pe|swap
                        acc = ps.tile([128, 512], F32, tag="mm", bufs=6)
                        for k in range(4):
                            nc.tensor.matmul(acc, wqb_t[:, k, m * 128:(m + 1) * 128],
                                             cq_t[:, k, :], start=(k == 0), stop=(k == 3))
                        h = m // 2
                        if m % 2 == 0:
                            nc.vector.tensor_copy(qn_t[:, h, sl], acc)
                        else:
                            pe_s = sb.tile([128, 512], BF16, tag="pes", bufs=2)
                            nc.scalar.copy(pe_s, acc)
                            t0 = sb.tile([DR, 512], BF16, tag="t0", bufs=2)
                            t1 = sb.tile([DR, 512], BF16, tag="t1", bufs=2)
                            nc.vector.tensor_mul(t0, pe_s[0:DR, :], cs_t[0:DR, sl])
                            nc.vector.tensor_mul(t1, pe_s[DR:128, :], cs_t[DR:128, sl])
                            nc.vector.tensor_add(qp_t[:, h, sl], t0, t1)
                    for m in range(2):  # k_nope per head
                        acc = ps.tile([128, 512], F32, tag="mm", bufs=6)
                        for k in range(4):
                            nc.tensor.matmul(acc, wkb_t[:, k, m * 128:(m + 1) * 128],
                                             ckv_t[:, k, :], start=(k == 0), stop=(k == 3))
                        nc.scalar.copy(kn_t[:, m, sl], acc)
                    for tp in range(2):  # v, token-major, two 128-token halves
                        acc = ps.tile([128, 2, 256], F32, tag="mm", bufs=6)
                        for t4 in range(2):
                            for k in range(4):
                                nc.tensor.matmul(
                                    acc[:, t4, :],
                                    ckv_t[:, k, (2 * tp + t4) * 128:(2 * tp + t4 + 1) * 128],
                                    wvb_t[:, k, :], start=(k == 0), stop=(k == 3))
                        nc.vector.tensor_copy(v_t[:, tt * 4 + 2 * tp:tt * 4 + 2 * tp + 2, :],
                                              acc)

                # ---- causal attention (scores transposed: [k, q]) ----
                def normalize(pend):
                    hh, den_p, oacc_p, qsl_p = pend
                    rec = sb.tile([128, 512], F32R, tag="rec", bufs=2)
                    with nc.allow_low_precision(reason="fp32r softmax denom"):
                        nc.vector.reciprocal(rec, den_p)
                    nc.vector.tensor_mul(o_t[:, hh, qsl_p], oacc_p, rec)

                wo_queue = []
                wo_out_box = [None]

                def wo_chunk(t16, ch):
                    tsl = slice(t16 * 128, (t16 + 1) * 128)
                    acc = ps.tile([128, 512], F32, tag="mm", bufs=6)
                    for hh in range(2):
                        nc.tensor.matmul(acc, o_t[:, hh, tsl],
                                         wop_t[:, hh, ch * 512:(ch + 1) * 512],
                                         start=(hh == 0), stop=(hh == 1))
                    if ch == 0:
                        outs = sb.tile([128, 2048], F32, tag="outs", bufs=2)
                        wo_out_box[0] = outs
                    else:
                        outs = wo_out_box[0]
                    if ch % 2 == 0:
                        nc.scalar.copy(outs[:, ch * 512:(ch + 1) * 512], acc)
                    else:
                        nc.vector.tensor_copy(outs[:, ch * 512:(ch + 1) * 512], acc)
                    if ch == 3:
                        nc.sync.dma_start(
                            out=out[b * S + t16 * 128:b * S + (t16 + 1) * 128, :],
                            in_=outs)

                pend_box = [None]

                def attn_qt(qt):
                    nonlocal wo_queue
                    for h in range(2):
                        qsl = slice(qt * 512, (qt + 1) * 512)
                        nkt = 4 * qt + 4
                        oacc = ps.tile([128, 512], F32, tag="pv", bufs=2)

                        # softmax denominators: exp tiles are zero-padded below
                        # their causal offset; DVE adds batch groups of
                        # ES_GROUP tiles and GPSIMD partition_all_reduce turns
                        # each group into broadcast column sums (fp32).
                        lst = {"pend": None, "es": None, "cnt": 0,
                               "den": None, "den_owned": False}

                        def close_group(last):
                            src = lst["es"] if lst["es"] is not None else lst["pend"]
                            deng = sb.tile([128, 512], F32, tag="deng", bufs=2)
                            nc.gpsimd.partition_all_reduce(deng, src, channels=128,
                                                           reduce_op=RADD)
                            if lst["den"] is None:
                                lst["den"] = deng
                            elif not lst["den_owned"]:
                                dacc = sb.tile([128, 512], F32, tag="den", bufs=2)
                                nc.vector.tensor_add(dacc, lst["den"], deng)
                                lst["den"] = dacc
                                lst["den_owned"] = True
                            else:
                                nc.vector.tensor_add(lst["den"], lst["den"], deng)
                            lst["pend"] = None
                            lst["es"] = None
                            lst["cnt"] = 0

                        def consume(prev_e):
                            et_p, off_p, kt_p = prev_e
                            last = (kt_p == nkt - 1)
                            nc.tensor.matmul(oacc[:, off_p:512],
                                             v_t[:, kt_p, h * 128:(h + 1) * 128],
                                             et_p[:, off_p:512],
                                             start=(kt_p == 0), stop=last)
                            if lst["pend"] is None and lst["es"] is None:
                                lst["pend"] = et_p
                                lst["cnt"] = 1
                            elif lst["es"] is None:
                                es = sb.tile([128, 512], BF16, tag="es", bufs=2)
                                nc.vector.tensor_add(es, lst["pend"], et_p)
                                lst["pend"] = None
                                lst["es"] = es
                                lst["cnt"] = 2
                            else:
                                nc.vector.tensor_add(lst["es"], lst["es"], et_p)
                                lst["cnt"] += 1
                            if last or lst["cnt"] >= ES_GROUP:
                                close_group(last)

                        prev = None
                        for kt in range(nkt):
                            ksl = slice(kt * 128, (kt + 1) * 128)
                            j = kt - 4 * qt
                            # columns of this q-tile that can be unmasked:
                            off = 128 * j if j > 0 else 0
                            w = 512 - off
                            qs2 = slice(qt * 512 + off, (qt + 1) * 512)
                            sc = ps.tile([128, 512], F32, tag="mm", bufs=6)
                            nc.tensor.matmul(sc[:, :w], kn_t[:, h, ksl],
                                             qn_t[:, h, qs2], start=True, stop=False)
                            nc.tensor.matmul(sc[:, :w], kp_t[:, ksl],
                                             qp_t[:, h, qs2], start=False, stop=True)
                            if prev is not None:
                                consume(prev)
                            et = sb.tile([128, 512], BF16, tag="exp", bufs=6)
                            if j > 0:
                                nc.gpsimd.memset(et[:, :off], 0.0)
                            nc.scalar.activation(et[:, off:512], sc[:, :w], AF.Exp,
                                                 scale=SCALE)
                            if 0 <= j < 4:
                                nc.vector.tensor_mul(et[:, off:off + 128],
                                                     et[:, off:off + 128], trim_t)
                            prev = (et, off, kt)
                            if kt == 0:
                                if pend_box[0] is not None:
                                    normalize(pend_box[0])
                                    pend_box[0] = None
                            elif wo_queue:
                                wo_chunk(*wo_queue.pop(0))
                        consume(prev)
                        pend_box[0] = (h, lst["den"], oacc, qsl)
                        if h == 1:
                            wo_queue += [(t16, ch) for t16 in
                                         range(qt * 4, qt * 4 + 4) for ch in range(4)]

                # software pipeline: proj tiles feed attention one tile ahead
                proj_tt(0)
                proj_tt(1)
                attn_qt(0)
                proj_tt(2)
                attn_qt(1)
                proj_tt(3)
                attn_qt(2)
                attn_qt(3)
                if pend_box[0] is not None:
                    normalize(pend_box[0])
                    pend_box[0] = None
                for t16, ch in wo_queue:
                    wo_chunk(t16, ch)

    nc.compile()
    return nc


# --------------------------------------------------------------------------
# Host-side data prep
# --------------------------------------------------------------------------
def _pack(wT, ktiles):
    """(ktiles*128, M) -> (128, ktiles, M) with [p, k, m] = wT[k*128+p, m]."""
    K, M = wT.shape
    assert K == ktiles * 128
    return np.ascontiguousarray(
        wT.reshape(ktiles, 128, M).transpose(1, 0, 2)).astype(NPBF)


def _swap_pairs(a, axis):
    idx = np.arange(a.shape[axis])
    idx = idx.reshape(-1, 2)[:, ::-1].reshape(-1)
    return np.take(a, idx, axis=axis)


def _prep(inputs):
    x = np.asarray(inputs["x"], dtype=np.float32)
    f = np.asarray(inputs["freqs_cis"], dtype=np.float32)
    wq_a = np.asarray(inputs["wq_a"], dtype=np.float32)
    wq_b = np.asarray(inputs["wq_b"], dtype=np.float32)
    q_norm_w = np.asarray(inputs["q_norm_w"], dtype=np.float32)
    wkv_a = np.asarray(inputs["wkv_a"], dtype=np.float32)
    kv_norm_w = np.asarray(inputs["kv_norm_w"], dtype=np.float32)
    wkv_b = np.asarray(inputs["wkv_b"], dtype=np.float32)
    wo = np.asarray(inputs["wo"], dtype=np.float32)

    xT = np.ascontiguousarray(x.reshape(T, DIM).T).astype(NPBF)  # (DIM, T)

    cos = f[:, :, 0].T  # (32, S)
    sin = f[:, :, 1].T
    cosF = np.empty((DR, S), np.float32)
    sinF = np.empty((DR, S), np.float32)
    cosF[0::2] = cos
    cosF[1::2] = cos
    sinF[0::2] = -sin
    sinF[1::2] = sin

    wqaT = wq_a.T                       # (DIM, RQ)
    wkvaT = wkv_a.T                     # (DIM, RKV+DR)
    pe = wkvaT[:, RKV:RKV + DR]
    wkva_ext = np.concatenate([wkvaT[:, :RKV], pe, _swap_pairs(pe, 1)], axis=1)
    wqa_p = _pack(wqaT, 16)
    wkva_p = _pack(wkva_ext, 16)

    csB = np.concatenate([cosF, sinF], axis=0).astype(NPBF)  # (128, S)

    k1_maps = []
    for c in range(NCORES):
        t0 = c * TS
        srange = slice(t0 % S, t0 % S + TS)
        k1_maps.append({
            "xt": np.ascontiguousarray(xT[:, t0:t0 + TS]),
            "wqa": wqa_p, "wkva": wkva_p,
            "csk": np.ascontiguousarray(csB[:, srange]),
        })

    # launch-2 per-core weights
    wqbT = (wq_b * q_norm_w[None, :]).T       # (RQ, H*DQK)
    wkvbT = (wkv_b * kv_norm_w[None, :]).T    # (RKV, H*(DN+DV))
    woT = wo.T                                # (H*DV, DIM)

    # strict-lower-triangle keep-mask for the 128x128 diagonal sub-block
    kk = np.arange(128)[:, None]
    qq = np.arange(128)[None, :]
    trim = (qq >= kk).astype(NPBF)

    csf = np.concatenate([cosF, sinF], axis=0).astype(NPBF)

    k2_maps = []
    for c in range(NCORES):
        h0, h1 = 2 * c, 2 * c + 1
        qcols = []
        for hh in (h0, h1):
            base = hh * DQK
            nope = wqbT[:, base:base + DN]
            pe_q = wqbT[:, base + DN:base + DQK]
            qcols += [nope, pe_q, _swap_pairs(pe_q, 1)]
        q_ext = np.concatenate(qcols, axis=1)             # (512, 512)
        kcols = [wkvbT[:, hh * (DN + DV):hh * (DN + DV) + DN] for hh in (h0, h1)]
        vcols = [wkvbT[:, hh * (DN + DV) + DN:(hh + 1) * (DN + DV)] for hh in (h0, h1)]
        worows = np.concatenate([woT[hh * DV:(hh + 1) * DV] for hh in (h0, h1)], axis=0)
        k2_maps.append({
            "wqb": _pack(q_ext, 4),
            "wkb": _pack(np.concatenate(kcols, axis=1), 4),
            "wvb": _pack(np.concatenate(vcols, axis=1), 4),
            "wop": _pack(worows, 2),
            "csf": csf, "trim": trim,
        })
    return k1_maps, k2_maps


def _get(name, builder):
    if name not in _CACHE:
        _CACHE[name] = builder()
    return _CACHE[name]


def _run(inputs, trace=False):
    k1_maps, k2_maps = _prep(inputs)
    nc1 = _get("k1", build_k1)
    r1 = run_bass_kernel_spmd(nc1, k1_maps, core_ids=list(range(NCORES)), trace=trace)

    cqn = np.concatenate([r1.results[c]["cqn"] for c in range(NCORES)], axis=1)
    ckvn = np.concatenate([r1.results[c]["ckvn"] for c in range(NCORES)], axis=1)
    kpe = np.concatenate([r1.results[c]["kpe"] for c in range(NCORES)], axis=1)
    for m in k2_maps:
        m["cqn"] = cqn
        m["ckvn"] = ckvn
        m["kpe"] = kpe

    nc2 = _get("k2", build_k2)
    r2 = run_bass_kernel_spmd(nc2, k2_maps, core_ids=list(range(NCORES)), trace=trace)

    acc = r2.results[0]["out"].astype(np.float32)
    for c in range(1, NCORES):
        acc = acc + r2.results[c]["out"]
    return acc.reshape(B, S, DIM), (r1, r2)


def kernel(**inputs) -> np.ndarray:
    out, _ = _run(inputs)
    return out


# revision 19
# speedup vs baseline: 1.2127x; 1.0311x over previous
"""MLA-style attention (nn_Attention_15496242004691) on 8 trn2 NeuronCores.

Strategy:
  Launch 1 (token-sharded, 512 tokens/core): cq = x@wq_a.T, ckv = x@wkv_a.T,
    RMSNorm of both (norm weights folded into the B projections on host),
    RoPE of k_pe (pair-swap folded into an extended wkv_a on host).
  Launch 2 (head-sharded, 2 heads/core): B projections (+q RoPE), causal
    attention with transposed scores (softmax column sums via GPSIMD
    partition_all_reduce over batched exp tiles), output projection; host
    sums the 8 partial outputs.

All tensors are bf16 except PSUM accumulation (fp32), the softmax
denominators (fp32) and the final output (fp32 partials summed on host).
Activations are kept feature-on-partition so no on-chip transposes occur.
DMAs are batched into few large transfers: descriptor generation (HWDGE) is
a serial ~625ns/DMA resource that a naive per-tile DMA schedule saturates.
"""

import numpy as np
import ml_dtypes

import concourse.bass as bass
import concourse.bass_isa as bass_isa
import concourse.mybir as mybir
import concourse.tile as tile
from concourse import bacc, library_config
from concourse.bass_utils import run_bass_kernel_spmd

F32 = mybir.dt.float32
F32R = mybir.dt.float32r
BF16 = mybir.dt.bfloat16
NPBF = ml_dtypes.bfloat16
AF = mybir.ActivationFunctionType
OP = mybir.AluOpType
RADD = bass_isa.ReduceOp.add

B, S, DIM, H = 2, 2048, 2048, 16
NCORES = 8
HPC = H // NCORES  # heads per core = 2
RQ = RKV = 512
DN, DR, DV, DQK = 128, 64, 128, 192
EPS = 1e-6
SCALE = DQK ** -0.5
T = B * S          # 4096 tokens
TS = T // NCORES   # 512 tokens per core in launch 1
ES_GROUP = 6       # exp tiles summed on DVE per partition_all_reduce

_CACHE = {}


# --------------------------------------------------------------------------
# Launch 1: A-projections + RMSNorm + k_pe RoPE (token-sharded)
# --------------------------------------------------------------------------
def build_k1():
    nc = bacc.Bacc("TRN2", target_bir_lowering=False)
    xt = nc.dram_tensor("xt", [DIM, TS], BF16, kind="ExternalInput")
    wqa = nc.dram_tensor("wqa", [128, 16, RQ], BF16, kind="ExternalInput")
    wkva = nc.dram_tensor("wkva", [128, 16, RKV + 2 * DR], BF16, kind="ExternalInput")
    csk = nc.dram_tensor("csk", [128, TS], BF16, kind="ExternalInput")
    cqn = nc.dram_tensor("cqn", [RQ, TS], BF16, kind="ExternalOutput")
    ckvn = nc.dram_tensor("ckvn", [RKV, TS], BF16, kind="ExternalOutput")
    kpe = nc.dram_tensor("kpe", [DR, TS], BF16, kind="ExternalOutput")

    cqn_r = cqn[:, :].rearrange("(m p) t -> p m t", p=128)
    ckvn_r = ckvn[:, :].rearrange("(m p) t -> p m t", p=128)

    with tile.TileContext(nc) as tc:
        with tc.tile_pool(name="const", bufs=1) as cpool, \
             tc.tile_pool(name="sb", bufs=2) as sb, \
             tc.tile_pool(name="ps", bufs=1, space="PSUM") as ps:
            nc.gpsimd.load_library(library_config.attn)
            eps_t = cpool.tile([128, 1], F32)
            nc.vector.memset(eps_t, EPS)

            xt_t = cpool.tile([128, 16, TS], BF16)
            xt_r = xt[:, :].rearrange("(k p) t -> p k t", p=128)
            wqa_t = cpool.tile([128, 16, RQ], BF16)
            wkva_t = cpool.tile([128, 16, RKV + 2 * DR], BF16)
            cs_t = cpool.tile([128, TS], BF16)
            # batched loads, split so the first matmuls can start early
            nc.sync.dma_start(out=wqa_t[:, 0:2, :], in_=wqa[:, 0:2, :])
            nc.sync.dma_start(out=xt_t[:, 0:2, :], in_=xt_r[:, 0:2, :])
            nc.sync.dma_start(out=wqa_t[:, 2:6, :], in_=wqa[:, 2:6, :])
            nc.sync.dma_start(out=xt_t[:, 2:6, :], in_=xt_r[:, 2:6, :])
            nc.sync.dma_start(out=wqa_t[:, 6:16, :], in_=wqa[:, 6:16, :])
            nc.sync.dma_start(out=xt_t[:, 6:16, :], in_=xt_r[:, 6:16, :])
            nc.sync.dma_start(out=wkva_t, in_=wkva[:, :, :])
            nc.sync.dma_start(out=cs_t, in_=csk[:, :])

            for path in ("q", "kv"):
                w_t = wqa_t if path == "q" else wkva_t
                out_r = cqn_r if path == "q" else ckvn_r
                nm = 4 if path == "q" else 5
                cqu = sb.tile([128, 4, TS], BF16, tag=f"cqu{path}", bufs=1)
                o_all = sb.tile([128, 4, TS], BF16, tag=f"oall{path}", bufs=1)
                pe_s = None
                sqs = []
                var_h = []
                for m in range(nm):
                    acc = ps.tile([128, TS], F32, tag="mm", bufs=3)
                    for k in range(16):
                        nc.tensor.matmul(acc, w_t[:, k, m * 128:(m + 1) * 128],
                                         xt_t[:, k, :], start=(k == 0), stop=(k == 15))
                    if m < 4:
                        sq = sb.tile([128, TS], BF16, tag="sq", bufs=2)
                        nc.scalar.activation(sq, acc, AF.Square)
                        nc.vector.tensor_copy(cqu[:, m, :], acc)
                        sqs.append(sq)
                        if m % 2 == 1:
                            # partition-reduce each half pair as soon as ready
                            ss = sb.tile([128, TS], BF16, tag="ss", bufs=2)
                            nc.vector.tensor_add(ss, sqs[-2], sqs[-1])
                            vh = sb.tile([128, TS], F32, tag="vh", bufs=2)
                            nc.gpsimd.partition_all_reduce(vh, ss, channels=128,
                                                           reduce_op=RADD)
                            var_h.append(vh)
                    else:
                        pe_s = sb.tile([128, TS], BF16, tag="pes", bufs=1)
                        nc.scalar.copy(pe_s, acc)
                # rsqrt(mean-square + eps), already broadcast across partitions
                var_bc = sb.tile([128, TS], F32, tag="var", bufs=2)
                nc.vector.tensor_add(var_bc, var_h[0], var_h[1])
                bcs = sb.tile([128, TS], BF16, tag="bcs", bufs=2)
                nc.scalar.activation(bcs, var_bc, AF.Abs_reciprocal_sqrt,
                                     scale=1.0 / 512.0, bias=eps_t[:, :])
                for m in range(4):
                    nc.vector.tensor_mul(o_all[:, m, :], cqu[:, m, :], bcs)
                    if m % 2 == 1:
                        nc.sync.dma_start(out=out_r[:, m - 1:m + 1, :],
                                          in_=o_all[:, m - 1:m + 1, :])
                if path == "kv":
                    # RoPE on k_pe: rows 0:64 = pe, 64:128 = pair-swapped pe
                    t0 = sb.tile([DR, TS], BF16, tag="t0", bufs=1)
                    t1 = sb.tile([DR, TS], BF16, tag="t1", bufs=1)
                    nc.vector.tensor_mul(t0, pe_s[0:DR, :], cs_t[0:DR, :])
                    nc.vector.tensor_mul(t1, pe_s[DR:128, :], cs_t[DR:128, :])
                    kp = sb.tile([DR, TS], BF16, tag="kp", bufs=1)
                    nc.vector.tensor_add(kp, t0, t1)
                    nc.sync.dma_start(out=kpe[:, :], in_=kp)
    nc.compile()
    return nc


# --------------------------------------------------------------------------
# Launch 2: B-projections + q RoPE + causal attention + wo (head-sharded)
# --------------------------------------------------------------------------
def build_k2():
    nc = bacc.Bacc("TRN2", target_bir_lowering=False)
    cqn = nc.dram_tensor("cqn", [RQ, T], BF16, kind="ExternalInput")
    ckvn = nc.dram_tensor("ckvn", [RKV, T], BF16, kind="ExternalInput")
    kpe = nc.dram_tensor("kpe", [DR, T], BF16, kind="ExternalInput")
    wqb = nc.dram_tensor("wqb", [128, 4, 512], BF16, kind="ExternalInput")
    wkb = nc.dram_tensor("wkb", [128, 4, 256], BF16, kind="ExternalInput")
    wvb = nc.dram_tensor("wvb", [128, 4, 256], BF16, kind="ExternalInput")
    wop = nc.dram_tensor("wop", [128, 2, DIM], BF16, kind="ExternalInput")
    csf = nc.dram_tensor("csf", [128, S], BF16, kind="ExternalInput")
    trim = nc.dram_tensor("trim", [128, 128], BF16, kind="ExternalInput")
    out = nc.dram_tensor("out", [T, DIM], F32, kind="ExternalOutput")

    cqn_r = cqn[:, :].rearrange("(k p) t -> p k t", p=128)
    ckvn_r = ckvn[:, :].rearrange("(k p) t -> p k t", p=128)

    with tile.TileContext(nc) as tc:
        with tc.tile_pool(name="const", bufs=1) as cpool, \
             tc.tile_pool(name="perb", bufs=1) as perb, \
             tc.tile_pool(name="sb", bufs=2) as sb, \
             tc.tile_pool(name="ps", bufs=1, space="PSUM") as ps:
            nc.gpsimd.load_library(library_config.attn)
            wqb_t = cpool.tile([128, 4, 512], BF16)
            wkb_t = cpool.tile([128, 4, 256], BF16)
            wvb_t = cpool.tile([128, 4, 256], BF16)
            wop_t = cpool.tile([128, 2, DIM], BF16)
            cs_t = cpool.tile([128, S], BF16)
            trim_t = cpool.tile([128, 128], BF16)

            consts_loaded = False

            # work deferred across batches so trailing output-projection
            # chunks of batch b overlap batch b+1's projections/attention
            wo_queue = []
            wo_out_box = [None]
            pend_box = [None]

            def normalize(pend):
                hh, den_p, oacc_p, qsl_p, o_ref = pend
                rec = sb.tile([128, 512], F32, tag="rec", bufs=2)
                nc.vector.reciprocal_approx_fast(rec, den_p)
                nc.vector.tensor_mul(o_ref[:, hh, qsl_p], oacc_p, rec)

            def wo_chunk(ent):
                bb, o_ref, t16, ch = ent
                tsl = slice(t16 * 128, (t16 + 1) * 128)
                acc = ps.tile([128, 512], F32, tag="mm", bufs=6)
                for hh in range(2):
                    nc.tensor.matmul(acc, o_ref[:, hh, tsl],
                                     wop_t[:, hh, ch * 512:(ch + 1) * 512],
                                     start=(hh == 0), stop=(hh == 1))
                if ch == 0:
                    outs = sb.tile([128, 2048], F32, tag="outs", bufs=2)
                    wo_out_box[0] = outs
                else:
                    outs = wo_out_box[0]
                if ch % 2 == 0:
                    nc.scalar.copy(outs[:, ch * 512:(ch + 1) * 512], acc)
                else:
                    nc.vector.tensor_copy(outs[:, ch * 512:(ch + 1) * 512], acc)
                if ch % 2 == 1:
                    half = slice((ch - 1) * 512, (ch + 1) * 512)
                    nc.sync.dma_start(
                        out=out[bb * S + t16 * 128:bb * S + (t16 + 1) * 128, half],
                        in_=outs[:, half])

            for b in range(B):
                qn_t = perb.tile([128, 2, S], BF16, tag="qn")
                qp_t = perb.tile([DR, 2, S], BF16, tag="qp")
                kn_t = perb.tile([128, 2, S], BF16, tag="kn")
                kp_t = perb.tile([DR, S], BF16, tag="kp")
                v_t = perb.tile([128, 16, 256], BF16, tag="v")
                o_t = perb.tile([128, 2, S], BF16, tag="o", bufs=2)

                # ---- B projection of one 512-token tile ----
                def proj_tt(tt):
                    nonlocal consts_loaded
                    g0 = b * S + tt * 512
                    sl = slice(tt * 512, (tt + 1) * 512)
                    cq_t = sb.tile([128, 4, 512], BF16, tag="cq", bufs=2)
                    ckv_t = sb.tile([128, 4, 512], BF16, tag="ckv", bufs=2)
                    if not consts_loaded:
                        # fine-grained first loads so the first matmul starts early
                        nc.sync.dma_start(out=wqb_t[:, 0, :], in_=wqb[:, 0, :])
                        nc.sync.dma_start(out=cq_t[:, 0, :], in_=cqn_r[:, 0, g0:g0 + 512])
                        nc.sync.dma_start(out=wqb_t[:, 1:4, :], in_=wqb[:, 1:4, :])
                        nc.sync.dma_start(out=cq_t[:, 1:4, :],
                                          in_=cqn_r[:, 1:4, g0:g0 + 512])
                    else:
                        nc.sync.dma_start(out=cq_t, in_=cqn_r[:, :, g0:g0 + 512])
                    nc.sync.dma_start(out=ckv_t, in_=ckvn_r[:, :, g0:g0 + 512])
                    if not consts_loaded:
                        nc.sync.dma_start(out=cs_t, in_=csf[:, :])
                        nc.sync.dma_start(out=wkb_t, in_=wkb[:, :, :])
                        nc.sync.dma_start(out=wvb_t, in_=wvb[:, :, :])
                        consts_loaded = True
                    elif tt == 1 and b == 0:
                        nc.sync.dma_start(out=trim_t, in_=trim[:, :])
                    elif tt == 2 and b == 0:
                        nc.sync.dma_start(out=wop_t, in_=wop[:, :, :])
                    if tt == 0:
                        nc.sync.dma_start(out=kp_t, in_=kpe[:, b * S:(b + 1) * S])

                    for m in range(4):  # h0 nope, h0 pe|swap, h1 nope, h1 pe|swap
                        acc = ps.tile([128, 512], F32, tag="mm", bufs=6)
                        for k in range(4):
                            nc.tensor.matmul(acc, wqb_t[:, k, m * 128:(m + 1) * 128],
                                             cq_t[:, k, :], start=(k == 0), stop=(k == 3))
                        h = m // 2
                        if m % 2 == 0:
                            nc.vector.tensor_copy(qn_t[:, h, sl], acc)
                        else:
                            pe_s = sb.tile([128, 512], BF16, tag="pes", bufs=2)
                            nc.scalar.copy(pe_s, acc)
                            t0 = sb.tile([DR, 512], BF16, tag="t0", bufs=2)
                            t1 = sb.tile([DR, 512], BF16, tag="t1", bufs=2)
                            nc.vector.tensor_mul(t0, pe_s[0:DR, :], cs_t[0:DR, sl])
                            nc.vector.tensor_mul(t1, pe_s[DR:128, :], cs_t[DR:128, sl])
                            nc.vector.tensor_add(qp_t[:, h, sl], t0, t1)
                    for m in range(2):  # k_nope per head
                        acc = ps.tile([128, 512], F32, tag="mm", bufs=6)
                        for k in range(4):
                            nc.tensor.matmul(acc, wkb_t[:, k, m * 128:(m + 1) * 128],
                                             ckv_t[:, k, :], start=(k == 0), stop=(k == 3))
                        nc.scalar.copy(kn_t[:, m, sl], acc)
                    for tp in range(2):  # v, token-major, two 128-token halves
                        acc = ps.tile([128, 2, 256], F32, tag="mm", bufs=6)
                        for t4 in range(2):
                            for k in range(4):
                                nc.tensor.matmul(
                                    acc[:, t4, :],
                                    ckv_t[:, k, (2 * tp + t4) * 128:(2 * tp + t4 + 1) * 128],
                                    wvb_t[:, k, :], start=(k == 0), stop=(k == 3))
                        nc.vector.tensor_copy(v_t[:, tt * 4 + 2 * tp:tt * 4 + 2 * tp + 2, :],
                                              acc)

                # ---- causal attention (scores transposed: [k, q]) ----
                def attn_qt(qt):
                    for h in range(2):
                        qsl = slice(qt * 512, (qt + 1) * 512)
                        nkt = 4 * qt + 4
                        oacc = ps.tile([128, 512], F32, tag="pv", bufs=2)

                        # softmax denominators: exp tiles are zero-padded below
                        # their causal offset; DVE adds batch groups of
                        # ES_GROUP tiles and GPSIMD partition_all_reduce turns
                        # each group into broadcast column sums (fp32).
                        lst = {"pend": None, "es": None, "cnt": 0,
                               "den": None, "den_owned": False}

                        def close_group(last):
                            src = lst["es"] if lst["es"] is not None else lst["pend"]
                            deng = sb.tile([128, 512], F32, tag="deng", bufs=2)
                            nc.gpsimd.partition_all_reduce(deng, src, channels=128,
                                                           reduce_op=RADD)
                            if lst["den"] is None:
                                lst["den"] = deng
                            elif not lst["den_owned"]:
                                dacc = sb.tile([128, 512], F32, tag="den", bufs=2)
                                nc.vector.tensor_add(dacc, lst["den"], deng)
                                lst["den"] = dacc
                                lst["den_owned"] = True
                            else:
                                nc.vector.tensor_add(lst["den"], lst["den"], deng)
                            lst["pend"] = None
                            lst["es"] = None
                            lst["cnt"] = 0

                        def consume(prev_e):
                            et_p, off_p, kt_p = prev_e
                            last = (kt_p == nkt - 1)
                            nc.tensor.matmul(oacc[:, off_p:512],
                                             v_t[:, kt_p, h * 128:(h + 1) * 128],
                                             et_p[:, off_p:512],
                                             start=(kt_p == 0), stop=last)
                            if lst["pend"] is None and lst["es"] is None:
                                lst["pend"] = et_p
                                lst["cnt"] = 1
                            elif lst["es"] is None:
                                es = sb.tile([128, 512], BF16, tag="es", bufs=2)
                                nc.vector.tensor_add(es, lst["pend"], et_p)
                                lst["pend"] = None
                                lst["es"] = es
                                lst["cnt"] = 2
                            else:
                                nc.vector.tensor_add(lst["es"], lst["es"], et_p)
                                lst["cnt"] += 1
                            if last or lst["cnt"] >= ES_GROUP:
                                close_group(last)

                        prev = None
                        for kt in range(nkt):
                            ksl = slice(kt * 128, (kt + 1) * 128)
                            j = kt - 4 * qt
                            # columns of this q-tile that can be unmasked:
                            off = 128 * j if j > 0 else 0
                            w = 512 - off
                            qs2 = slice(qt * 512 + off, (qt + 1) * 512)
                            sc = ps.tile([128, 512], F32, tag="mm", bufs=6)
                            nc.tensor.matmul(sc[:, :w], kn_t[:, h, ksl],
                                             qn_t[:, h, qs2], start=True, stop=False)
                            nc.tensor.matmul(sc[:, :w], kp_t[:, ksl],
                                             qp_t[:, h, qs2], start=False, stop=True)
                            if prev is not None:
                                consume(prev)
                            et = sb.tile([128, 512], BF16, tag="exp", bufs=6)
                            if j > 0:
                                nc.gpsimd.memset(et[:, :off], 0.0)
                            nc.scalar.activation(et[:, off:512], sc[:, :w], AF.Exp,
                                                 scale=SCALE)
                            if 0 <= j < 4:
                                nc.vector.tensor_mul(et[:, off:off + 128],
                                                     et[:, off:off + 128], trim_t)
                            prev = (et, off, kt)
                            if kt == 0:
                                if pend_box[0] is not None:
                                    normalize(pend_box[0])
                                    pend_box[0] = None
                            elif kt >= 2 and wo_queue:
                                wo_chunk(wo_queue.pop(0))
                        consume(prev)
                        pend_box[0] = (h, lst["den"], oacc, qsl, o_t)
                        if h == 1:
                            wo_queue.extend([(b, o_t, t16, ch) for t16 in
                                             range(qt * 4, qt * 4 + 4) for ch in range(4)])

                # software pipeline: proj tiles feed attention one tile ahead
                proj_tt(0)
                proj_tt(1)
                attn_qt(0)
                proj_tt(2)
                attn_qt(1)
                proj_tt(3)
                attn_qt(2)
                attn_qt(3)

            if pend_box[0] is not None:
                normalize(pend_box[0])
                pend_box[0] = None
            for ent in wo_queue:
                wo_chunk(ent)

    nc.compile()
    return nc


# --------------------------------------------------------------------------
# Host-side data prep
# --------------------------------------------------------------------------
def _pack(wT, ktiles):
    """(ktiles*128, M) -> (128, ktiles, M) with [p, k, m] = wT[k*128+p, m]."""
    K, M = wT.shape
    assert K == ktiles * 128
    return np.ascontiguousarray(
        wT.reshape(ktiles, 128, M).transpose(1, 0, 2)).astype(NPBF)


def _swap_pairs(a, axis):
    idx = np.arange(a.shape[axis])
    idx = idx.reshape(-1, 2)[:, ::-1].reshape(-1)
    return np.take(a, idx, axis=axis)


def _prep(inputs):
    x = np.asarray(inputs["x"], dtype=np.float32)
    f = np.asarray(inputs["freqs_cis"], dtype=np.float32)
    wq_a = np.asarray(inputs["wq_a"], dtype=np.float32)
    wq_b = np.asarray(inputs["wq_b"], dtype=np.float32)
    q_norm_w = np.asarray(inputs["q_norm_w"], dtype=np.float32)
    wkv_a = np.asarray(inputs["wkv_a"], dtype=np.float32)
    kv_norm_w = np.asarray(inputs["kv_norm_w"], dtype=np.float32)
    wkv_b = np.asarray(inputs["wkv_b"], dtype=np.float32)
    wo = np.asarray(inputs["wo"], dtype=np.float32)

    xT = np.ascontiguousarray(x.reshape(T, DIM).T).astype(NPBF)  # (DIM, T)

    cos = f[:, :, 0].T  # (32, S)
    sin = f[:, :, 1].T
    cosF = np.empty((DR, S), np.float32)
    sinF = np.empty((DR, S), np.float32)
    cosF[0::2] = cos
    cosF[1::2] = cos
    sinF[0::2] = -sin
    sinF[1::2] = sin

    wqaT = wq_a.T                       # (DIM, RQ)
    wkvaT = wkv_a.T                     # (DIM, RKV+DR)
    pe = wkvaT[:, RKV:RKV + DR]
    wkva_ext = np.concatenate([wkvaT[:, :RKV], pe, _swap_pairs(pe, 1)], axis=1)
    wqa_p = _pack(wqaT, 16)
    wkva_p = _pack(wkva_ext, 16)

    csB = np.concatenate([cosF, sinF], axis=0).astype(NPBF)  # (128, S)

    k1_maps = []
    for c in range(NCORES):
        t0 = c * TS
        srange = slice(t0 % S, t0 % S + TS)
        k1_maps.append({
            "xt": np.ascontiguousarray(xT[:, t0:t0 + TS]),
            "wqa": wqa_p, "wkva": wkva_p,
            "csk": np.ascontiguousarray(csB[:, srange]),
        })

    # launch-2 per-core weights
    wqbT = (wq_b * q_norm_w[None, :]).T       # (RQ, H*DQK)
    wkvbT = (wkv_b * kv_norm_w[None, :]).T    # (RKV, H*(DN+DV))
    woT = wo.T                                # (H*DV, DIM)

    # strict-lower-triangle keep-mask for the 128x128 diagonal sub-block
    kk = np.arange(128)[:, None]
    qq = np.arange(128)[None, :]
    trim = (qq >= kk).astype(NPBF)

    csf = np.concatenate([cosF, sinF], axis=0).astype(NPBF)

    k2_maps = []
    for c in range(NCORES):
        h0, h1 = 2 * c, 2 * c + 1
        qcols = []
        for hh in (h0, h1):
            base = hh * DQK
            nope = wqbT[:, base:base + DN]
            pe_q = wqbT[:, base + DN:base + DQK]
            qcols += [nope, pe_q, _swap_pairs(pe_q, 1)]
        q_ext = np.concatenate(qcols, axis=1)             # (512, 512)
        kcols = [wkvbT[:, hh * (DN + DV):hh * (DN + DV) + DN] for hh in (h0, h1)]
        vcols = [wkvbT[:, hh * (DN + DV) + DN:(hh + 1) * (DN + DV)] for hh in (h0, h1)]
        worows = np.concatenate([woT[hh * DV:(hh + 1) * DV] for hh in (h0, h1)], axis=0)
        k2_maps.append({
            "wqb": _pack(q_ext, 4),
            "wkb": _pack(np.concatenate(kcols, axis=1), 4),
            "wvb": _pack(np.concatenate(vcols, axis=1), 4),
            "wop": _pack(worows, 2),
            "csf": csf, "trim": trim,
        })
    return k1_maps, k2_maps


def _get(name, builder):
    if name not in _CACHE:
        _CACHE[name] = builder()
    return _CACHE[name]


def _run(inputs, trace=False):
    k1_maps, k2_maps = _prep(inputs)
    nc1 = _get("k1", build_k1)
    r1 = run_bass_kernel_spmd(nc1, k1_maps, core_ids=list(range(NCORES)), trace=trace)

    cqn = np.concatenate([r1.results[c]["cqn"] for c in range(NCORES)], axis=1)
    ckvn = np.concatenate([r1.results[c]["ckvn"] for c in range(NCORES)], axis=1)
    kpe = np.concatenate([r1.results[c]["kpe"] for c in range(NCORES)], axis=1)
    for m in k2_maps:
        m["cqn"] = cqn
        m["ckvn"] = ckvn
        m["kpe"] = kpe

    nc2 = _get("k2", build_k2)
    r2 = run_bass_kernel_spmd(nc2, k2_maps, core_ids=list(range(NCORES)), trace=trace)

    acc = r2.results[0]["out"].astype(np.float32)
    for c in range(1, NCORES):
        acc = acc + r2.results[c]["out"]
    return acc.reshape(B, S, DIM), (r1, r2)


def kernel(**inputs) -> np.ndarray:
    out, _ = _run(inputs)
    return out
